# revision 18
# baseline (speedup 1.0000x reference)
"""DiagonalStateElman Trainium2 kernel.

Math (per batch row b, data-parallel over 8 cores):
    xz   = x @ W_in^T                       -> split x_proj, z
    u_t  = silu(x_proj_t) @ B_mat           (h-independent, precomputable)
    h_t  = tanh(A*h_{t-1} + u_t)            (elementwise, A = sigmoid(log_A))
    y_t  = (h_t @ C_mat) * silu(z_t)
    out  = y @ W_out^T

Key trick: A = sigmoid(log_A) < 1 and |tanh'| <= 1, so the recurrence forgets
its state geometrically (factor max(A) per step).  T is split into chunks of
K=64 tokens; every chunk restarts from zero state W=32 tokens early
(max(A)^W < 2^-26, below fp32 resolution) so all chunks of a T/2 half run in
lockstep: 96 sequential steps of one wide vector op instead of 2048.

The recurrence of each half overlaps the tensor engine's projection work of
the other half, so wall time ~= pure matmul time.  Everything is transposed
(features on partitions, time on free axis) so the four matmuls chain without
transposes.  fp16 operands, fp32 PSUM accumulate; the u buffer is stored
k-major so the per-step strided gather is contiguous.
"""

import numpy as np

import concourse.bass as bass
import concourse.tile as tile
import concourse.bacc as bacc
import concourse.mybir as mybir
from concourse.bass_utils import run_bass_kernel_spmd

P = 128
T = 2048
DIM = 1024
DI = 1024          # d_inner
DS = 2048          # d_state
NCORES = 8
TS = 512           # time-slab for projection phases
K = 64             # chunk length
W = 32             # warmup tokens (also the front-pad inside block 0)
F16 = mybir.dt.float16
F32 = mybir.dt.float32

DG = DIM // P      # 8  dim groups (contraction for XZ)
EG = 2 * DI // P   # 16 e-chunks of XZ output
SG = DS // P       # 16 d_state groups
JG = DI // P       # 8  d_inner groups
OG = DIM // P      # 8  output dim groups
TH = T // 2        # tokens per half
CH = TH // K       # 16 chunks per half
NC_ = CH + 1       # u-buffer blocks per half (1 leading warmup block)
NSLAB = T // TS    # 4 slabs (2 per half)
STEPS = K + W      # 96 lockstep steps per half
IB = 32            # macro-2 step-block (IB*CH = 512 = one PSUM bank)

LAST_RESULT = None  # stashed BassKernelResults (for the test harness)


def _build(a_const):
    """Emit + compile the Bass program. a_const: float A (constant) or None."""
    mult = mybir.AluOpType.mult
    add = mybir.AluOpType.add
    Sigmoid = mybir.ActivationFunctionType.Sigmoid
    Tanh = mybir.ActivationFunctionType.Tanh

    nc = bacc.Bacc("TRN2", target_bir_lowering=False, debug=False,
                   num_devices=NCORES)

    xT_d = nc.dram_tensor("xT", [DIM, T], F16, kind="ExternalInput").ap()
    w_inT_d = nc.dram_tensor("w_inT", [DIM, 2 * DI], F16, kind="ExternalInput").ap()
    b_d = nc.dram_tensor("b_mat", [DI, DS], F16, kind="ExternalInput").ap()
    c_d = nc.dram_tensor("c_mat", [DS, DI], F16, kind="ExternalInput").ap()
    w_outT_d = nc.dram_tensor("w_outT", [DI, DIM], F16, kind="ExternalInput").ap()
    ah0_d = nc.dram_tensor("ah0", [P, SG], F32, kind="ExternalInput").ap()
    a_d = None
    if a_const is None:
        a_d = nc.dram_tensor("a_vec", [P, SG], F32, kind="ExternalInput").ap()
    out_hd = [nc.dram_tensor(f"out_rec{h}", [P, OG, K, CH], F32,
                             kind="ExternalOutput").ap() for h in range(2)]
    hfin_d = nc.dram_tensor("h_fin", [P, SG], F32, kind="ExternalOutput").ap()

    xT_v = xT_d.rearrange("(g p) t -> p g t", p=P)
    w_inT_v = w_inT_d.rearrange("(g p) e -> p g e", p=P)
    b_v = b_d.rearrange("(g p) s -> p g s", p=P)
    c_v = c_d.rearrange("(g p) j -> p g j", p=P)
    w_outT_v = w_outT_d.rearrange("(g p) o -> p g o", p=P)

    with tile.TileContext(nc) as tc:
        with tc.tile_pool(name="dram", bufs=1, space="DRAM") as dpool, \
             tc.tile_pool(name="psum", bufs=6, space="PSUM") as ppool, \
             tc.tile_pool(name="st", bufs=3) as st, \
             tc.tile_pool(name="hst", bufs=2) as hst, \
             tc.tile_pool(name="u1p", bufs=1) as u1p, \
             tc.tile_pool(name="ahp", bufs=1) as ahp:

            gz_dd = [dpool.tile([P, JG, TH], F16, tag=f"gz_d{h}",
                                name=f"gz_d{h}") for h in range(2)]
            # H in recurrence order, 8-step blocks: [p, g, i8, c]
            hrec = [[dpool.tile([P, SG, 8, CH], F16, tag=f"hrec{h}_{i}",
                                name=f"hrec{h}_{i}") for i in range(K // 8)]
                    for h in range(2)]

            ah0_sb = ahp.tile([P, SG], F32, tag="ah0")
            nc.sync.dma_start(out=ah0_sb, in_=ah0_d)
            a_sb = None
            if a_const is None:
                a_sb = ahp.tile([P, SG], F32, tag="a_sb")
                nc.sync.dma_start(out=a_sb, in_=a_d)

            # u buffers, k-major: [p, k, g, c_blk]; col (c_blk*K + k) holds
            # u(token_local = c_blk*K + k - K); block 0 k>=W is the leading
            # warmup pad of the half.  U[1] outlives the macro-1 pools (read
            # by the half-1 recurrence that overlaps macro 2).
            U = [None, u1p.tile([P, K, SG, NC_], F16, tag="U1", name="U1")]

            # ---------------- projection phases (per 512-token slab) -------
            def emit_slab(sl, ws, xs, sxp, gzp, sgp):
                h = sl // 2
                sloc = sl % 2
                t0 = sl * TS
                xslab = xs.tile([P, DG, TS], F16, tag="x", name=f"x{sl}")
                nc.sync.dma_start(out=xslab, in_=xT_v[:, :, t0:t0 + TS])
                sxs = sxp.tile([P, DG, TS], F16, tag="sx", name=f"sx{sl}")
                gzs = gzp.tile([P, JG, TS], F16, tag="gz", name=f"gz{sl}")
                for ec2 in range(EG // 2):
                    wt = ws.tile([P, DG, 2 * P], F16, tag="w", name=f"w{sl}_{ec2}")
                    nc.sync.dma_start(
                        out=wt, in_=w_inT_v[:, :, ec2 * 2 * P:(ec2 + 1) * 2 * P])
                    for half in range(2):
                        ec = ec2 * 2 + half
                        ps = ppool.tile([P, TS], F32, tag="mm", name=f"psA{sl}_{ec}")
                        for g in range(DG):
                            nc.tensor.matmul(
                                ps, lhsT=wt[:, g, half * P:(half + 1) * P],
                                rhs=xslab[:, g, :],
                                start=(g == 0), stop=(g == DG - 1))
                        tgt = sxs[:, ec, :] if ec < DG else gzs[:, ec - DG, :]
                        # silu(v) = v * sigmoid(v)
                        sg = sgp.tile([P, TS], F16, tag="sg", name=f"sg{sl}_{ec}")
                        nc.scalar.activation(out=sg, in_=ps, func=Sigmoid)
                        nc.vector.tensor_tensor(out=tgt, in0=ps, in1=sg, op=mult)
                nc.sync.dma_start(
                    out=gz_dd[h][:, :, sloc * TS:(sloc + 1) * TS], in_=gzs)
                for sc2 in range(SG // 2):
                    bt = ws.tile([P, DG, 2 * P], F16, tag="b", name=f"b{sl}_{sc2}")
                    nc.sync.dma_start(
                        out=bt, in_=b_v[:, :, sc2 * 2 * P:(sc2 + 1) * 2 * P])
                    for half in range(2):
                        sc = sc2 * 2 + half
                        ps = ppool.tile([P, TS], F32, tag="mm", name=f"psB{sl}_{sc}")
                        for g in range(DG):
                            nc.tensor.matmul(
                                ps, lhsT=bt[:, g, half * P:(half + 1) * P],
                                rhs=sxs[:, g, :],
                                start=(g == 0), stop=(g == DG - 1))
                        # transposed store: psum col j=(c*K+k) -> U[k, sc, c0+c]
                        c0 = 1 + sloc * (TS // K)
                        nc.vector.tensor_copy(
                            out=U[h][:, :, sc, c0:c0 + TS // K],
                            in_=ps.rearrange("p (c k) -> p k c", k=K))
                        if sl == 1:
                            # warmup pad of half 1 = last W tokens of half 0
                            nc.vector.tensor_copy(
                                out=U[1][:, W:K, sc, 0], in_=ps[:, TS - W:TS])

            # -------------- lockstep chunked recurrence for one half -------
            def emit_rec(h):
                GH = SG // 2
                Sc = []
                for ch in range(2):
                    s0 = st.tile([P, GH, CH], F16, tag=f"s{ch}",
                                 name=f"s0_{h}_{ch}")
                    nc.vector.memset(s0, 0.0)
                    Sc.append(s0)
                hstage = None
                for i in range(STEPS):
                    r = i - W
                    if r >= 0 and r % 8 == 0:
                        hstage = hst.tile([P, SG, 8, CH], F16, tag="hst",
                                          name=f"hs{h}_{r // 8}")
                    kk = i + W
                    c_off = kk // K
                    k = kk % K
                    for ch in range(2):
                        g0 = ch * GH
                        u = U[h][:, k, g0:g0 + GH, c_off:c_off + CH]
                        t1 = st.tile([P, GH, CH], F32, tag=f"t{ch}",
                                     name=f"t{h}_{i}_{ch}")
                        if a_const is not None:
                            nc.vector.scalar_tensor_tensor(
                                out=t1, in0=Sc[ch], scalar=a_const, in1=u,
                                op0=mult, op1=add)
                        else:
                            nc.vector.tensor_tensor(
                                out=t1, in0=Sc[ch],
                                in1=a_sb[:, g0:g0 + GH, None].to_broadcast(
                                    [P, GH, CH]), op=mult)
                            nc.vector.tensor_tensor(out=t1, in0=t1, in1=u, op=add)
                        if r >= 0:
                            s1 = hstage[:, g0:g0 + GH, r % 8, :]
                        else:
                            s1 = st.tile([P, GH, CH], F16, tag=f"s{ch}",
                                         name=f"sw{h}_{i}_{ch}")
                        nc.scalar.activation(out=s1, in_=t1, func=Tanh)
                        Sc[ch] = s1
                        if h == 1 and i == STEPS - 1:
                            hf = st.tile([P, GH, CH], F32, tag=f"hf{ch}",
                                         name=f"hf{ch}", bufs=1)
                            nc.scalar.activation(out=hf, in_=t1, func=Tanh)
                            nc.sync.dma_start(
                                out=hfin_d[:, g0:g0 + GH],
                                in_=hf[:, :, CH - 1])
                    if r >= 0 and r % 8 == 7:
                        nc.sync.dma_start(out=hrec[h][r // 8], in_=hstage)

            # -------------- macro 2 for one half: Y, gate, out -------------
            def emit_m2(h, c_sb, wout_sb, gzsbp, hsp, gp, op_):
                gz_sb = gzsbp.tile([P, JG, TH], F16, tag=f"gzsb{h}",
                                   name=f"gzsb{h}")
                nc.sync.dma_start(out=gz_sb, in_=gz_dd[h])
                for ib in range(K // IB):
                    hsl = hsp.tile([P, SG, IB, CH], F16, tag="hsl",
                                   name=f"hsl{h}_{ib}")
                    for q in range(IB // 8):
                        nc.sync.dma_start(
                            out=hsl[:, :, q * 8:(q + 1) * 8, :],
                            in_=hrec[h][ib * (IB // 8) + q])
                    hflat = hsl.rearrange("p g a b -> p g (a b)")
                    Gt = gp.tile([P, JG, IB * CH], F16, tag="G",
                                 name=f"G{h}_{ib}")
                    for jc in range(JG):
                        ps = ppool.tile([P, IB * CH], F32, tag="mm",
                                        name=f"psY{h}_{ib}_{jc}")
                        for g in range(SG):
                            nc.tensor.matmul(
                                ps, lhsT=c_sb[:, g, jc * P:(jc + 1) * P],
                                rhs=hflat[:, g, :],
                                start=(g == 0), stop=(g == SG - 1))
                        gzv = gz_sb[:, jc, :].rearrange(
                            "p (c i) -> p i c", i=K)[:, ib * IB:(ib + 1) * IB, :]
                        nc.vector.tensor_tensor(
                            out=Gt[:, jc, :].rearrange("p (a b) -> p a b", b=CH),
                            in0=ps.rearrange("p (a b) -> p a b", b=CH),
                            in1=gzv, op=mult)
                    for oc in range(OG):
                        ps2 = ppool.tile([P, IB * CH], F32, tag="mm",
                                         name=f"psO{h}_{ib}_{oc}")
                        for jc in range(JG):
                            nc.tensor.matmul(
                                ps2, lhsT=wout_sb[:, jc, oc * P:(oc + 1) * P],
                                rhs=Gt[:, jc, :],
                                start=(jc == 0), stop=(jc == JG - 1))
                        ost = op_.tile([P, IB * CH], F32, tag="ost",
                                       name=f"ost{h}_{ib}_{oc}")
                        nc.vector.tensor_copy(out=ost, in_=ps2)
                        nc.sync.dma_start(
                            out=out_hd[h][:, oc, ib * IB:(ib + 1) * IB, :],
                            in_=ost.rearrange("p (a b) -> p a b", b=CH))

            # ------------------------- schedule ----------------------------
            with tc.tile_pool(name="m2w", bufs=1) as m2w:
                with tc.tile_pool(name="u0p", bufs=1) as u0p, \
                     tc.tile_pool(name="ws", bufs=2) as ws, \
                     tc.tile_pool(name="xs", bufs=2) as xs, \
                     tc.tile_pool(name="sxp", bufs=1) as sxp, \
                     tc.tile_pool(name="gzp", bufs=1) as gzp, \
                     tc.tile_pool(name="sgp", bufs=2) as sgp:
                    U[0] = u0p.tile([P, K, SG, NC_], F16, tag="U0", name="U0")
                    nc.vector.memset(U[0][:, W:K, :, 0], 0.0)

                    emit_slab(0, ws, xs, sxp, gzp, sgp)
                    emit_slab(1, ws, xs, sxp, gzp, sgp)
                    # exact h0 handling: u(token 0) += A*h0  (col K of half 0)
                    nc.vector.tensor_tensor(
                        out=U[0][:, 0, :, 1], in0=U[0][:, 0, :, 1], in1=ah0_sb,
                        op=add)
                    emit_rec(0)
                    emit_slab(2, ws, xs, sxp, gzp, sgp)
                    # macro-2 weights load while slabs 2/3 + rec-h0 run
                    c_sb = m2w.tile([P, SG, DI], F16, tag="c_sb")
                    nc.sync.dma_start(out=c_sb, in_=c_v)
                    wout_sb = m2w.tile([P, JG, DIM], F16, tag="wout_sb")
                    nc.sync.dma_start(out=wout_sb, in_=w_outT_v)
                    emit_slab(3, ws, xs, sxp, gzp, sgp)

                with tc.tile_pool(name="gzsbp", bufs=1) as gzsbp, \
                     tc.tile_pool(name="hsp", bufs=2) as hsp, \
                     tc.tile_pool(name="gp", bufs=2) as gp, \
                     tc.tile_pool(name="op", bufs=3) as op_:
                    emit_m2(0, c_sb, wout_sb, gzsbp, hsp, gp, op_)
                    emit_rec(1)
                    emit_m2(1, c_sb, wout_sb, gzsbp, hsp, gp, op_)

    nc.compile()
    return nc


def kernel(x, h0, W_in, W_out, B_mat, C_mat, log_A):
    global LAST_RESULT
    x = np.asarray(x, dtype=np.float32)
    h0 = np.asarray(h0, dtype=np.float32)
    W_in = np.asarray(W_in, dtype=np.float32)
    W_out = np.asarray(W_out, dtype=np.float32)
    B_mat = np.asarray(B_mat, dtype=np.float32)
    C_mat = np.asarray(C_mat, dtype=np.float32)
    log_A = np.asarray(log_A, dtype=np.float32)

    B = x.shape[0]
    assert x.shape == (NCORES, T, DIM), x.shape

    A = 1.0 / (1.0 + np.exp(-log_A.astype(np.float64)))
    A = A.astype(np.float32)
    a_const = float(A[0]) if np.all(A == A[0]) else None
    amax = float(A.max())
    if amax > 0.0 and (amax >= 1.0 or -26.0 / np.log2(amax) > W):
        raise NotImplementedError(
            f"A too close to 1 for warmup {W} (amax={amax})")

    nc = _build(a_const)

    w_inT16 = np.ascontiguousarray(W_in.T).astype(np.float16)
    b16 = np.ascontiguousarray(B_mat).astype(np.float16)
    c16 = np.ascontiguousarray(C_mat).astype(np.float16)
    w_outT16 = np.ascontiguousarray(W_out.T).astype(np.float16)

    in_maps = []
    for b in range(B):
        m = {
            "xT": np.ascontiguousarray(x[b].T).astype(np.float16),
            "w_inT": w_inT16,
            "b_mat": b16,
            "c_mat": c16,
            "w_outT": w_outT16,
            "ah0": np.ascontiguousarray((A * h0[b]).reshape(SG, P).T).astype(
                np.float32),
        }
        if a_const is None:
            m["a_vec"] = np.ascontiguousarray(A.reshape(SG, P).T).astype(
                np.float32)
        in_maps.append(m)

    res = run_bass_kernel_spmd(nc, in_maps, core_ids=list(range(NCORES)))
    LAST_RESULT = res

    out = np.empty((B, T, DIM), dtype=np.float32)
    h_fin = np.empty((B, DS), dtype=np.float32)
    for b in range(B):
        for h in range(2):
            orec = np.asarray(res.results[b][f"out_rec{h}"]).reshape(
                P, OG, K, CH)
            out[b, h * TH:(h + 1) * TH] = orec.transpose(3, 2, 1, 0).reshape(
                TH, DIM)
        h_fin[b] = np.asarray(res.results[b]["h_fin"]).reshape(P, SG).T.reshape(DS)
    return out, h_fin


# revision 21
# speedup vs baseline: 1.0605x; 1.0605x over previous
"""DiagonalStateElman Trainium2 kernel.

Math (per batch row b, data-parallel over 8 cores):
    xz   = x @ W_in^T                       -> split x_proj, z
    u_t  = silu(x_proj_t) @ B_mat           (h-independent, precomputable)
    h_t  = tanh(A*h_{t-1} + u_t)            (elementwise, A = sigmoid(log_A))
    y_t  = (h_t @ C_mat) * silu(z_t)
    out  = y @ W_out^T

Key trick: A = sigmoid(log_A) < 1 and |tanh'| <= 1, so the recurrence forgets
its state geometrically (factor max(A) per step).  T is split into chunks of
K=64 tokens; every chunk restarts from zero state W=32 tokens early
(max(A)^W < 2^-26, below fp32 resolution) so all chunks of a T/2 half run in
lockstep: 96 sequential steps of one wide vector op instead of 2048.

The recurrence of each half overlaps the tensor engine's projection work of
the other half, so wall time ~= pure matmul time.  Everything is transposed
(features on partitions, time on free axis) so the four matmuls chain without
transposes.  fp16 operands, fp32 PSUM accumulate; the u buffer is stored
k-major so the per-step strided gather is contiguous.
"""

import numpy as np

import concourse.bass as bass
import concourse.tile as tile
import concourse.bacc as bacc
import concourse.mybir as mybir
from concourse.bass_utils import run_bass_kernel_spmd

P = 128
T = 2048
DIM = 1024
DI = 1024          # d_inner
DS = 2048          # d_state
NCORES = 8
TS = 512           # time-slab for projection phases
K = 64             # chunk length
W = 32             # warmup tokens (also the front-pad inside block 0)
F16 = mybir.dt.float16
F32 = mybir.dt.float32

DG = DIM // P      # 8  dim groups (contraction for XZ)
EG = 2 * DI // P   # 16 e-chunks of XZ output
SG = DS // P       # 16 d_state groups
JG = DI // P       # 8  d_inner groups
OG = DIM // P      # 8  output dim groups
TH = T // 2        # tokens per half
CH = TH // K       # 16 chunks per half
NC_ = CH + 1       # u-buffer blocks per half (1 leading warmup block)
NSLAB = T // TS    # 4 slabs (2 per half)
STEPS = K + W      # 96 lockstep steps per half
IB = 32            # macro-2 step-block (IB*CH = 512 = one PSUM bank)

LAST_RESULT = None  # stashed BassKernelResults (for the test harness)


def _build(a_const):
    """Emit + compile the Bass program. a_const: float A (constant) or None."""
    mult = mybir.AluOpType.mult
    add = mybir.AluOpType.add
    Sigmoid = mybir.ActivationFunctionType.Sigmoid
    Tanh = mybir.ActivationFunctionType.Tanh

    nc = bacc.Bacc("TRN2", target_bir_lowering=False, debug=False,
                   num_devices=NCORES)

    xT_d = nc.dram_tensor("xT", [DIM, T], F16, kind="ExternalInput").ap()
    w_inT_d = nc.dram_tensor("w_inT", [DIM, 2 * DI], F16, kind="ExternalInput").ap()
    b_d = nc.dram_tensor("b_mat", [DI, DS], F16, kind="ExternalInput").ap()
    c_d = nc.dram_tensor("c_mat", [DS, DI], F16, kind="ExternalInput").ap()
    w_outT_d = nc.dram_tensor("w_outT", [DI, DIM], F16, kind="ExternalInput").ap()
    ah0_d = nc.dram_tensor("ah0", [P, SG], F32, kind="ExternalInput").ap()
    a_d = None
    if a_const is None:
        a_d = nc.dram_tensor("a_vec", [P, SG], F32, kind="ExternalInput").ap()
    out_hd = [nc.dram_tensor(f"out_rec{h}", [P, OG, K, CH], F32,
                             kind="ExternalOutput").ap() for h in range(2)]
    hfin_d = nc.dram_tensor("h_fin", [P, SG], F32, kind="ExternalOutput").ap()

    xT_v = xT_d.rearrange("(g p) t -> p g t", p=P)
    w_inT_v = w_inT_d.rearrange("(g p) e -> p g e", p=P)
    b_v = b_d.rearrange("(g p) s -> p g s", p=P)
    c_v = c_d.rearrange("(g p) j -> p g j", p=P)
    w_outT_v = w_outT_d.rearrange("(g p) o -> p g o", p=P)

    with tile.TileContext(nc) as tc:
        with tc.tile_pool(name="dram", bufs=1, space="DRAM") as dpool, \
             tc.tile_pool(name="psum", bufs=6, space="PSUM") as ppool, \
             tc.tile_pool(name="st", bufs=3) as st, \
             tc.tile_pool(name="hst", bufs=2) as hst, \
             tc.tile_pool(name="u1p", bufs=1) as u1p, \
             tc.tile_pool(name="ahp", bufs=1) as ahp:

            gz_dd = [dpool.tile([P, JG, TH], F16, tag=f"gz_d{h}",
                                name=f"gz_d{h}") for h in range(2)]
            # H in recurrence order, 8-step blocks: [p, g, i8, c]
            hrec = [[dpool.tile([P, SG, 8, CH], F16, tag=f"hrec{h}_{i}",
                                name=f"hrec{h}_{i}") for i in range(K // 8)]
                    for h in range(2)]

            ah0_sb = ahp.tile([P, SG], F32, tag="ah0")
            nc.sync.dma_start(out=ah0_sb, in_=ah0_d)
            a_sb = None
            if a_const is None:
                a_sb = ahp.tile([P, SG], F32, tag="a_sb")
                nc.sync.dma_start(out=a_sb, in_=a_d)

            # u buffers, k-major: [p, k, g, c_blk]; col (c_blk*K + k) holds
            # u(token_local = c_blk*K + k - K); block 0 k>=W is the leading
            # warmup pad of the half.  U[1] outlives the macro-1 pools (read
            # by the half-1 recurrence that overlaps macro 2).
            U = [None, u1p.tile([P, K, SG, NC_], F16, tag="U1", name="U1")]

            # ---------------- projection phases (per 512-token slab) -------
            def emit_slab(sl, ws, xs, sxp, gzp, sgp, b_sb):
                h = sl // 2
                sloc = sl % 2
                t0 = sl * TS
                xslab = xs.tile([P, DG, TS], F16, tag="x", name=f"x{sl}")
                nc.sync.dma_start(out=xslab, in_=xT_v[:, :, t0:t0 + TS])
                sxs = sxp.tile([P, DG, TS], F16, tag="sx", name=f"sx{sl}")
                gzs = gzp.tile([P, JG, TS], F16, tag="gz", name=f"gz{sl}")
                for ec2 in range(EG // 2):
                    wt = ws.tile([P, DG, 2 * P], F16, tag="w", name=f"w{sl}_{ec2}")
                    nc.sync.dma_start(
                        out=wt, in_=w_inT_v[:, :, ec2 * 2 * P:(ec2 + 1) * 2 * P])
                    for half in range(2):
                        ec = ec2 * 2 + half
                        ps = ppool.tile([P, TS], F32, tag="mm", name=f"psA{sl}_{ec}")
                        for g in range(DG):
                            nc.tensor.matmul(
                                ps, lhsT=wt[:, g, half * P:(half + 1) * P],
                                rhs=xslab[:, g, :],
                                start=(g == 0), stop=(g == DG - 1))
                        tgt = sxs[:, ec, :] if ec < DG else gzs[:, ec - DG, :]
                        # silu(v) = v * sigmoid(v)
                        sg = sgp.tile([P, TS], F16, tag="sg", name=f"sg{sl}_{ec}")
                        nc.scalar.activation(out=sg, in_=ps, func=Sigmoid)
                        nc.vector.tensor_tensor(out=tgt, in0=ps, in1=sg, op=mult)
                nc.sync.dma_start(
                    out=gz_dd[h][:, :, sloc * TS:(sloc + 1) * TS], in_=gzs)
                for sc in range(SG):
                    ps = ppool.tile([P, TS], F32, tag="mm", name=f"psB{sl}_{sc}")
                    for g in range(DG):
                        nc.tensor.matmul(
                            ps, lhsT=b_sb[:, g, sc * P:(sc + 1) * P],
                            rhs=sxs[:, g, :],
                            start=(g == 0), stop=(g == DG - 1))
                    # transposed store: psum col j=(c*K+k) -> U[k, sc, c0+c]
                    c0 = 1 + sloc * (TS // K)
                    nc.vector.tensor_copy(
                        out=U[h][:, :, sc, c0:c0 + TS // K],
                        in_=ps.rearrange("p (c k) -> p k c", k=K))
                    if sl == 1:
                        # warmup pad of half 1 = last W tokens of half 0
                        nc.vector.tensor_copy(
                            out=U[1][:, W:K, sc, 0], in_=ps[:, TS - W:TS])

            # -------------- lockstep chunked recurrence for one half -------
            def emit_rec(h):
                GH = SG // 2
                Sc = []
                for ch in range(2):
                    s0 = st.tile([P, GH, CH], F16, tag=f"s{ch}",
                                 name=f"s0_{h}_{ch}")
                    nc.vector.memset(s0, 0.0)
                    Sc.append(s0)
                hstage = None
                for i in range(STEPS):
                    r = i - W
                    if r >= 0 and r % 8 == 0:
                        hstage = hst.tile([P, SG, 8, CH], F16, tag="hst",
                                          name=f"hs{h}_{r // 8}")
                    kk = i + W
                    c_off = kk // K
                    k = kk % K
                    for ch in range(2):
                        g0 = ch * GH
                        u = U[h][:, k, g0:g0 + GH, c_off:c_off + CH]
                        t1 = st.tile([P, GH, CH], F32, tag=f"t{ch}",
                                     name=f"t{h}_{i}_{ch}")
                        if a_const is not None:
                            nc.vector.scalar_tensor_tensor(
                                out=t1, in0=Sc[ch], scalar=a_const, in1=u,
                                op0=mult, op1=add)
                        else:
                            nc.vector.tensor_tensor(
                                out=t1, in0=Sc[ch],
                                in1=a_sb[:, g0:g0 + GH, None].to_broadcast(
                                    [P, GH, CH]), op=mult)
                            nc.vector.tensor_tensor(out=t1, in0=t1, in1=u, op=add)
                        if r >= 0:
                            s1 = hstage[:, g0:g0 + GH, r % 8, :]
                        else:
                            s1 = st.tile([P, GH, CH], F16, tag=f"s{ch}",
                                         name=f"sw{h}_{i}_{ch}")
                        nc.scalar.activation(out=s1, in_=t1, func=Tanh)
                        Sc[ch] = s1
                        if h == 1 and i == STEPS - 1:
                            hf = st.tile([P, GH, CH], F32, tag=f"hf{ch}",
                                         name=f"hf{ch}", bufs=1)
                            nc.scalar.activation(out=hf, in_=t1, func=Tanh)
                            nc.sync.dma_start(
                                out=hfin_d[:, g0:g0 + GH],
                                in_=hf[:, :, CH - 1])
                    if r >= 0 and r % 8 == 7:
                        nc.sync.dma_start(out=hrec[h][r // 8], in_=hstage)

            # -------------- macro 2 for one half: Y, gate, out -------------
            def emit_m2(h, c_sb, wout_sb, gzsbp, hsp, gp, op_):
                gz_sb = gzsbp.tile([P, JG, TH], F16, tag=f"gzsb{h}",
                                   name=f"gzsb{h}")
                nc.sync.dma_start(out=gz_sb, in_=gz_dd[h])
                for ib in range(K // IB):
                    hsl = hsp.tile([P, SG, IB, CH], F16, tag="hsl",
                                   name=f"hsl{h}_{ib}")
                    for q in range(IB // 8):
                        nc.sync.dma_start(
                            out=hsl[:, :, q * 8:(q + 1) * 8, :],
                            in_=hrec[h][ib * (IB // 8) + q])
                    hflat = hsl.rearrange("p g a b -> p g (a b)")
                    Gt = gp.tile([P, JG, IB * CH], F16, tag="G",
                                 name=f"G{h}_{ib}")
                    for jc in range(JG):
                        ps = ppool.tile([P, IB * CH], F32, tag="mm",
                                        name=f"psY{h}_{ib}_{jc}")
                        for g in range(SG):
                            nc.tensor.matmul(
                                ps, lhsT=c_sb[:, g, jc * P:(jc + 1) * P],
                                rhs=hflat[:, g, :],
                                start=(g == 0), stop=(g == SG - 1))
                        gzv = gz_sb[:, jc, :].rearrange(
                            "p (c i) -> p i c", i=K)[:, ib * IB:(ib + 1) * IB, :]
                        nc.vector.tensor_tensor(
                            out=Gt[:, jc, :].rearrange("p (a b) -> p a b", b=CH),
                            in0=ps.rearrange("p (a b) -> p a b", b=CH),
                            in1=gzv, op=mult)
                    for oc in range(OG):
                        ps2 = ppool.tile([P, IB * CH], F32, tag="mm",
                                         name=f"psO{h}_{ib}_{oc}")
                        for jc in range(JG):
                            nc.tensor.matmul(
                                ps2, lhsT=wout_sb[:, jc, oc * P:(oc + 1) * P],
                                rhs=Gt[:, jc, :],
                                start=(jc == 0), stop=(jc == JG - 1))
                        ost = op_.tile([P, IB * CH], F32, tag="ost",
                                       name=f"ost{h}_{ib}_{oc}")
                        nc.vector.tensor_copy(out=ost, in_=ps2)
                        nc.sync.dma_start(
                            out=out_hd[h][:, oc, ib * IB:(ib + 1) * IB, :],
                            in_=ost.rearrange("p (a b) -> p a b", b=CH))

            # ------------------------- schedule ----------------------------
            with tc.tile_pool(name="m2w", bufs=1) as m2w:
                with tc.tile_pool(name="u0p", bufs=1) as u0p, \
                     tc.tile_pool(name="m1", bufs=1) as m1, \
                     tc.tile_pool(name="ws", bufs=2) as ws, \
                     tc.tile_pool(name="xs", bufs=2) as xs, \
                     tc.tile_pool(name="sxp", bufs=1) as sxp, \
                     tc.tile_pool(name="gzp", bufs=1) as gzp, \
                     tc.tile_pool(name="sgp", bufs=2) as sgp:
                    U[0] = u0p.tile([P, K, SG, NC_], F16, tag="U0", name="U0")
                    nc.vector.memset(U[0][:, W:K, :, 0], 0.0)
                    b_sb = m1.tile([P, DG, DS], F16, tag="b_sb")
                    nc.sync.dma_start(out=b_sb, in_=b_v)

                    emit_slab(0, ws, xs, sxp, gzp, sgp, b_sb)
                    emit_slab(1, ws, xs, sxp, gzp, sgp, b_sb)
                    # exact h0 handling: u(token 0) += A*h0  (col K of half 0)
                    nc.vector.tensor_tensor(
                        out=U[0][:, 0, :, 1], in0=U[0][:, 0, :, 1], in1=ah0_sb,
                        op=add)
                    emit_rec(0)
                    emit_slab(2, ws, xs, sxp, gzp, sgp, b_sb)
                    # macro-2 C weights load while slabs 2/3 + rec-h0 run
                    c_sb = m2w.tile([P, SG, DI], F16, tag="c_sb")
                    nc.sync.dma_start(out=c_sb, in_=c_v)
                    emit_slab(3, ws, xs, sxp, gzp, sgp, b_sb)

                with tc.tile_pool(name="m2b", bufs=1) as m2b, \
                     tc.tile_pool(name="hsp", bufs=2) as hsp, \
                     tc.tile_pool(name="gp", bufs=2) as gp, \
                     tc.tile_pool(name="op", bufs=3) as op_:
                    wout_sb = m2b.tile([P, JG, DIM], F16, tag="wout_sb")
                    nc.sync.dma_start(out=wout_sb, in_=w_outT_v)
                    emit_m2(0, c_sb, wout_sb, m2b, hsp, gp, op_)
                    emit_rec(1)
                    emit_m2(1, c_sb, wout_sb, m2b, hsp, gp, op_)

    nc.compile()
    return nc


def kernel(x, h0, W_in, W_out, B_mat, C_mat, log_A):
    global LAST_RESULT
    x = np.asarray(x, dtype=np.float32)
    h0 = np.asarray(h0, dtype=np.float32)
    W_in = np.asarray(W_in, dtype=np.float32)
    W_out = np.asarray(W_out, dtype=np.float32)
    B_mat = np.asarray(B_mat, dtype=np.float32)
    C_mat = np.asarray(C_mat, dtype=np.float32)
    log_A = np.asarray(log_A, dtype=np.float32)

    B = x.shape[0]
    assert x.shape == (NCORES, T, DIM), x.shape

    A = 1.0 / (1.0 + np.exp(-log_A.astype(np.float64)))
    A = A.astype(np.float32)
    a_const = float(A[0]) if np.all(A == A[0]) else None
    amax = float(A.max())
    if amax > 0.0 and (amax >= 1.0 or -26.0 / np.log2(amax) > W):
        raise NotImplementedError(
            f"A too close to 1 for warmup {W} (amax={amax})")

    nc = _build(a_const)

    w_inT16 = np.ascontiguousarray(W_in.T).astype(np.float16)
    b16 = np.ascontiguousarray(B_mat).astype(np.float16)
    c16 = np.ascontiguousarray(C_mat).astype(np.float16)
    w_outT16 = np.ascontiguousarray(W_out.T).astype(np.float16)

    in_maps = []
    for b in range(B):
        m = {
            "xT": np.ascontiguousarray(x[b].T).astype(np.float16),
            "w_inT": w_inT16,
            "b_mat": b16,
            "c_mat": c16,
            "w_outT": w_outT16,
            "ah0": np.ascontiguousarray((A * h0[b]).reshape(SG, P).T).astype(
                np.float32),
        }
        if a_const is None:
            m["a_vec"] = np.ascontiguousarray(A.reshape(SG, P).T).astype(
                np.float32)
        in_maps.append(m)

    res = run_bass_kernel_spmd(nc, in_maps, core_ids=list(range(NCORES)))
    LAST_RESULT = res

    out = np.empty((B, T, DIM), dtype=np.float32)
    h_fin = np.empty((B, DS), dtype=np.float32)
    for b in range(B):
        for h in range(2):
            orec = np.asarray(res.results[b][f"out_rec{h}"]).reshape(
                P, OG, K, CH)
            out[b, h * TH:(h + 1) * TH] = orec.transpose(3, 2, 1, 0).reshape(
                TH, DIM)
        h_fin[b] = np.asarray(res.results[b]["h_fin"]).reshape(P, SG).T.reshape(DS)
    return out, h_fin


# revision 25
# speedup vs baseline: 1.1198x; 1.0560x over previous
"""DiagonalStateElman Trainium2 kernel.

Math (per batch row b, data-parallel over 8 cores):
    xz   = x @ W_in^T                       -> split x_proj, z
    u_t  = silu(x_proj_t) @ B_mat           (h-independent, precomputable)
    h_t  = tanh(A*h_{t-1} + u_t)            (elementwise, A = sigmoid(log_A))
    y_t  = (h_t @ C_mat) * silu(z_t)
    out  = y @ W_out^T

Key trick: A = sigmoid(log_A) < 1 and |tanh'| <= 1, so the recurrence forgets
its state geometrically (factor max(A) per step).  T is split into chunks of
K=64 tokens; every chunk restarts from zero state W=32 tokens early
(max(A)^W < 2^-26, below fp32 resolution) so all chunks of a T/2 half run in
lockstep: 96 sequential steps of one wide vector op instead of 2048.

The recurrence of each half overlaps the tensor engine's projection work of
the other half, so wall time ~= pure matmul time.  Everything is transposed
(features on partitions, time on free axis) so the four matmuls chain without
transposes.  fp16 operands, fp32 PSUM accumulate; the u buffer is stored
k-major so the per-step strided gather is contiguous.
"""

import numpy as np

import concourse.bass as bass
import concourse.tile as tile
import concourse.bacc as bacc
import concourse.mybir as mybir
from concourse.bass_utils import run_bass_kernel_spmd

P = 128
T = 2048
DIM = 1024
DI = 1024          # d_inner
DS = 2048          # d_state
NCORES = 8
TS = 512           # time-slab for projection phases
K = 64             # chunk length
W = 32             # warmup tokens (also the front-pad inside block 0)
F16 = mybir.dt.float16
F32 = mybir.dt.float32

DG = DIM // P      # 8  dim groups (contraction for XZ)
EG = 2 * DI // P   # 16 e-chunks of XZ output
SG = DS // P       # 16 d_state groups
JG = DI // P       # 8  d_inner groups
OG = DIM // P      # 8  output dim groups
TH = T // 2        # tokens per half
CH = TH // K       # 16 chunks per half
NC_ = CH + 1       # u-buffer blocks per half (1 leading warmup block)
NSLAB = T // TS    # 4 slabs (2 per half)
STEPS = K + W      # 96 lockstep steps per half
IB = 32            # macro-2 step-block (IB*CH = 512 = one PSUM bank)

LAST_RESULT = None  # stashed BassKernelResults (for the test harness)


def _build(a_const):
    """Emit + compile the Bass program. a_const: float A (constant) or None."""
    mult = mybir.AluOpType.mult
    add = mybir.AluOpType.add
    Sigmoid = mybir.ActivationFunctionType.Sigmoid
    Tanh = mybir.ActivationFunctionType.Tanh

    nc = bacc.Bacc("TRN2", target_bir_lowering=False, debug=False,
                   num_devices=NCORES)

    xT_d = nc.dram_tensor("xT", [DIM, T], F16, kind="ExternalInput").ap()
    w_inT_d = nc.dram_tensor("w_inT", [DIM, 2 * DI], F16, kind="ExternalInput").ap()
    b_d = nc.dram_tensor("b_mat", [DI, DS], F16, kind="ExternalInput").ap()
    c_d = nc.dram_tensor("c_mat", [DS, DI], F16, kind="ExternalInput").ap()
    w_outT_d = nc.dram_tensor("w_outT", [DI, DIM], F16, kind="ExternalInput").ap()
    ah0_d = nc.dram_tensor("ah0", [P, SG], F32, kind="ExternalInput").ap()
    a_d = None
    if a_const is None:
        a_d = nc.dram_tensor("a_vec", [P, SG], F32, kind="ExternalInput").ap()
    out_hd = [nc.dram_tensor(f"out_rec{h}", [P, OG, K, CH], F32,
                             kind="ExternalOutput").ap() for h in range(2)]
    hfin_d = nc.dram_tensor("h_fin", [P, SG], F32, kind="ExternalOutput").ap()

    xT_v = xT_d.rearrange("(g p) t -> p g t", p=P)
    w_inT_v = w_inT_d.rearrange("(g p) e -> p g e", p=P)
    b_v = b_d.rearrange("(g p) s -> p g s", p=P)
    c_v = c_d.rearrange("(g p) j -> p g j", p=P)
    w_outT_v = w_outT_d.rearrange("(g p) o -> p g o", p=P)

    with tile.TileContext(nc) as tc:
        with tc.tile_pool(name="dram", bufs=1, space="DRAM") as dpool, \
             tc.tile_pool(name="psum", bufs=6, space="PSUM") as ppool, \
             tc.tile_pool(name="st", bufs=3) as st, \
             tc.tile_pool(name="hst", bufs=2) as hst, \
             tc.tile_pool(name="u1p", bufs=1) as u1p, \
             tc.tile_pool(name="ahp", bufs=1) as ahp:

            gz_dd = [dpool.tile([P, JG, TH], F16, tag=f"gz_d{h}",
                                name=f"gz_d{h}") for h in range(2)]
            # H in recurrence order, 8-step blocks: [p, g, i8, c]
            hrec = [[dpool.tile([P, SG, 8, CH], F16, tag=f"hrec{h}_{i}",
                                name=f"hrec{h}_{i}") for i in range(K // 8)]
                    for h in range(2)]

            ah0_sb = ahp.tile([P, SG], F32, tag="ah0")
            nc.sync.dma_start(out=ah0_sb, in_=ah0_d)
            a_sb = None
            if a_const is None:
                a_sb = ahp.tile([P, SG], F32, tag="a_sb")
                nc.sync.dma_start(out=a_sb, in_=a_d)

            # u buffers, k-major: [p, k, g, c_blk]; col (c_blk*K + k) holds
            # u(token_local = c_blk*K + k - K); block 0 k>=W is the leading
            # warmup pad of the half.  U[1] outlives the macro-1 pools (read
            # by the half-1 recurrence that overlaps macro 2).
            U = [None, u1p.tile([P, K, SG, NC_], F16, tag="U1", name="U1")]

            # ---------------- projection phases (per 512-token slab) -------
            def emit_slab(sl, ws, xs, sxp, gzp, sgp, b_sb):
                h = sl // 2
                sloc = sl % 2
                t0 = sl * TS
                xslab = xs.tile([P, DG, TS], F16, tag="x", name=f"x{sl}")
                nc.sync.dma_start(out=xslab, in_=xT_v[:, :, t0:t0 + TS])
                sxs = sxp.tile([P, DG, TS], F16, tag="sx", name=f"sx{sl}")
                gzs = gzp.tile([P, JG, TS], F16, tag="gz", name=f"gz{sl}")
                for ec2 in range(EG // 2):
                    wt = ws.tile([P, DG, 2 * P], F16, tag="w", name=f"w{sl}_{ec2}")
                    nc.sync.dma_start(
                        out=wt, in_=w_inT_v[:, :, ec2 * 2 * P:(ec2 + 1) * 2 * P])
                    for half in range(2):
                        ec = ec2 * 2 + half
                        ps = ppool.tile([P, TS], F32, tag="mm", name=f"psA{sl}_{ec}")
                        for g in range(DG):
                            nc.tensor.matmul(
                                ps, lhsT=wt[:, g, half * P:(half + 1) * P],
                                rhs=xslab[:, g, :],
                                start=(g == 0), stop=(g == DG - 1))
                        tgt = sxs[:, ec, :] if ec < DG else gzs[:, ec - DG, :]
                        # silu(v) = v * sigmoid(v)
                        sg = sgp.tile([P, TS], F16, tag="sg", name=f"sg{sl}_{ec}")
                        nc.scalar.activation(out=sg, in_=ps, func=Sigmoid)
                        nc.vector.tensor_tensor(out=tgt, in0=ps, in1=sg, op=mult)
                nc.sync.dma_start(
                    out=gz_dd[h][:, :, sloc * TS:(sloc + 1) * TS], in_=gzs)
                for sc in range(SG):
                    ps = ppool.tile([P, TS], F32, tag="mm", name=f"psB{sl}_{sc}")
                    for g in range(DG):
                        nc.tensor.matmul(
                            ps, lhsT=b_sb[:, g, sc * P:(sc + 1) * P],
                            rhs=sxs[:, g, :],
                            start=(g == 0), stop=(g == DG - 1))
                    # transposed store: psum col j=(c*K+k) -> U[k, sc, c0+c]
                    c0 = 1 + sloc * (TS // K)
                    nc.vector.tensor_copy(
                        out=U[h][:, :, sc, c0:c0 + TS // K],
                        in_=ps.rearrange("p (c k) -> p k c", k=K))
                    if sl == 1:
                        # warmup pad of half 1 = last W tokens of half 0
                        nc.vector.tensor_copy(
                            out=U[1][:, W:K, sc, 0], in_=ps[:, TS - W:TS])

            # -------------- lockstep chunked recurrence for one half -------
            def emit_rec(h):
                GH = SG // 2
                Sc = []
                for ch in range(2):
                    s0 = st.tile([P, GH, CH], F16, tag=f"s{ch}",
                                 name=f"s0_{h}_{ch}")
                    nc.vector.memset(s0, 0.0)
                    Sc.append(s0)
                hstage = None
                for i in range(STEPS):
                    r = i - W
                    if r >= 0 and r % 8 == 0:
                        hstage = hst.tile([P, SG, 8, CH], F16, tag="hst",
                                          name=f"hs{h}_{r // 8}")
                    kk = i + W
                    c_off = kk // K
                    k = kk % K
                    for ch in range(2):
                        g0 = ch * GH
                        u = U[h][:, k, g0:g0 + GH, c_off:c_off + CH]
                        t1 = st.tile([P, GH, CH], F32, tag=f"t{ch}",
                                     name=f"t{h}_{i}_{ch}")
                        if a_const is not None:
                            nc.vector.scalar_tensor_tensor(
                                out=t1, in0=Sc[ch], scalar=a_const, in1=u,
                                op0=mult, op1=add)
                        else:
                            nc.vector.tensor_tensor(
                                out=t1, in0=Sc[ch],
                                in1=a_sb[:, g0:g0 + GH, None].to_broadcast(
                                    [P, GH, CH]), op=mult)
                            nc.vector.tensor_tensor(out=t1, in0=t1, in1=u, op=add)
                        if r >= 0:
                            s1 = hstage[:, g0:g0 + GH, r % 8, :]
                        else:
                            s1 = st.tile([P, GH, CH], F16, tag=f"s{ch}",
                                         name=f"sw{h}_{i}_{ch}")
                        nc.scalar.activation(out=s1, in_=t1, func=Tanh)
                        Sc[ch] = s1
                        if h == 1 and i == STEPS - 1:
                            hf = st.tile([P, GH, CH], F32, tag=f"hf{ch}",
                                         name=f"hf{ch}", bufs=1)
                            nc.scalar.activation(out=hf, in_=t1, func=Tanh)
                            nc.gpsimd.dma_start(
                                out=hfin_d[:, g0:g0 + GH],
                                in_=hf[:, :, CH - 1])
                    if r >= 0 and r % 8 == 7:
                        nc.sync.dma_start(out=hrec[h][r // 8], in_=hstage)

            # -------------- macro 2 for one half: Y, gate, out -------------
            def emit_m2(h, c_sb, wout_sb, gz_sb, hsp, gp, op_):
                for ib in range(K // IB):
                    hsl = hsp.tile([P, SG, IB, CH], F16, tag="hsl",
                                   name=f"hsl{h}_{ib}")
                    for q in range(IB // 8):
                        nc.sync.dma_start(
                            out=hsl[:, :, q * 8:(q + 1) * 8, :],
                            in_=hrec[h][ib * (IB // 8) + q])
                    hflat = hsl.rearrange("p g a b -> p g (a b)")
                    Gt = gp.tile([P, JG, IB * CH], F16, tag="G",
                                 name=f"G{h}_{ib}")
                    for jc in range(JG):
                        ps = ppool.tile([P, IB * CH], F32, tag="mm",
                                        name=f"psY{h}_{ib}_{jc}")
                        for g in range(SG):
                            nc.tensor.matmul(
                                ps, lhsT=c_sb[:, g, jc * P:(jc + 1) * P],
                                rhs=hflat[:, g, :],
                                start=(g == 0), stop=(g == SG - 1))
                        gzv = gz_sb[:, jc, :].rearrange(
                            "p (c i) -> p i c", i=K)[:, ib * IB:(ib + 1) * IB, :]
                        nc.vector.tensor_tensor(
                            out=Gt[:, jc, :].rearrange("p (a b) -> p a b", b=CH),
                            in0=ps.rearrange("p (a b) -> p a b", b=CH),
                            in1=gzv, op=mult)
                    for oc in range(OG):
                        ps2 = ppool.tile([P, IB * CH], F32, tag="mm",
                                         name=f"psO{h}_{ib}_{oc}")
                        for jc in range(JG):
                            nc.tensor.matmul(
                                ps2, lhsT=wout_sb[:, jc, oc * P:(oc + 1) * P],
                                rhs=Gt[:, jc, :],
                                start=(jc == 0), stop=(jc == JG - 1))
                        ost = op_.tile([P, IB * CH], F32, tag="ost",
                                       name=f"ost{h}_{ib}_{oc}")
                        nc.vector.tensor_copy(out=ost, in_=ps2)
                        nc.gpsimd.dma_start(
                            out=out_hd[h][:, oc, ib * IB:(ib + 1) * IB, :],
                            in_=ost.rearrange("p (a b) -> p a b", b=CH))

            # ------------------------- schedule ----------------------------
            with tc.tile_pool(name="m2w", bufs=1) as m2w:
                with tc.tile_pool(name="u0p", bufs=1) as u0p, \
                     tc.tile_pool(name="m1", bufs=1) as m1, \
                     tc.tile_pool(name="ws", bufs=3) as ws, \
                     tc.tile_pool(name="xs", bufs=2) as xs, \
                     tc.tile_pool(name="sxp", bufs=1) as sxp, \
                     tc.tile_pool(name="gzp", bufs=1) as gzp, \
                     tc.tile_pool(name="sgp", bufs=2) as sgp:
                    U[0] = u0p.tile([P, K, SG, NC_], F16, tag="U0", name="U0")
                    nc.vector.memset(U[0][:, W:K, :, 0], 0.0)
                    b_sb = m1.tile([P, DG, DS], F16, tag="b_sb")
                    nc.sync.dma_start(out=b_sb, in_=b_v)

                    emit_slab(0, ws, xs, sxp, gzp, sgp, b_sb)
                    emit_slab(1, ws, xs, sxp, gzp, sgp, b_sb)
                    # gz for half 0: load back early, hidden under slab 2/3
                    gz_sb0 = m2w.tile([P, JG, TH], F16, tag="gzsb0",
                                      name="gzsb0")
                    nc.sync.dma_start(out=gz_sb0, in_=gz_dd[0])
                    # exact h0 handling: u(token 0) += A*h0  (col K of half 0)
                    nc.vector.tensor_tensor(
                        out=U[0][:, 0, :, 1], in0=U[0][:, 0, :, 1], in1=ah0_sb,
                        op=add)
                    emit_rec(0)
                    emit_slab(2, ws, xs, sxp, gzp, sgp, b_sb)
                    # macro-2 C weights load while slabs 2/3 + rec-h0 run
                    c_sb = m2w.tile([P, SG, DI], F16, tag="c_sb")
                    nc.sync.dma_start(out=c_sb, in_=c_v)
                    emit_slab(3, ws, xs, sxp, gzp, sgp, b_sb)

                with tc.tile_pool(name="m2b", bufs=1) as m2b, \
                     tc.tile_pool(name="hsp", bufs=2) as hsp, \
                     tc.tile_pool(name="gp", bufs=2) as gp, \
                     tc.tile_pool(name="op", bufs=3) as op_:
                    wout_sb = m2b.tile([P, JG, DIM], F16, tag="wout_sb")
                    nc.scalar.dma_start(out=wout_sb, in_=w_outT_v)
                    emit_m2(0, c_sb, wout_sb, gz_sb0, hsp, gp, op_)
                    gz_sb1 = m2b.tile([P, JG, TH], F16, tag="gzsb1",
                                      name="gzsb1")
                    nc.sync.dma_start(out=gz_sb1, in_=gz_dd[1])
                    emit_rec(1)
                    emit_m2(1, c_sb, wout_sb, gz_sb1, hsp, gp, op_)

    nc.compile()
    return nc


def kernel(x, h0, W_in, W_out, B_mat, C_mat, log_A):
    global LAST_RESULT
    x = np.asarray(x, dtype=np.float32)
    h0 = np.asarray(h0, dtype=np.float32)
    W_in = np.asarray(W_in, dtype=np.float32)
    W_out = np.asarray(W_out, dtype=np.float32)
    B_mat = np.asarray(B_mat, dtype=np.float32)
    C_mat = np.asarray(C_mat, dtype=np.float32)
    log_A = np.asarray(log_A, dtype=np.float32)

    B = x.shape[0]
    assert x.shape == (NCORES, T, DIM), x.shape

    A = 1.0 / (1.0 + np.exp(-log_A.astype(np.float64)))
    A = A.astype(np.float32)
    a_const = float(A[0]) if np.all(A == A[0]) else None
    amax = float(A.max())
    if amax > 0.0 and (amax >= 1.0 or -26.0 / np.log2(amax) > W):
        raise NotImplementedError(
            f"A too close to 1 for warmup {W} (amax={amax})")

    nc = _build(a_const)

    w_inT16 = np.ascontiguousarray(W_in.T).astype(np.float16)
    b16 = np.ascontiguousarray(B_mat).astype(np.float16)
    c16 = np.ascontiguousarray(C_mat).astype(np.float16)
    w_outT16 = np.ascontiguousarray(W_out.T).astype(np.float16)

    in_maps = []
    for b in range(B):
        m = {
            "xT": np.ascontiguousarray(x[b].T).astype(np.float16),
            "w_inT": w_inT16,
            "b_mat": b16,
            "c_mat": c16,
            "w_outT": w_outT16,
            "ah0": np.ascontiguousarray((A * h0[b]).reshape(SG, P).T).astype(
                np.float32),
        }
        if a_const is None:
            m["a_vec"] = np.ascontiguousarray(A.reshape(SG, P).T).astype(
                np.float32)
        in_maps.append(m)

    res = run_bass_kernel_spmd(nc, in_maps, core_ids=list(range(NCORES)))
    LAST_RESULT = res

    out = np.empty((B, T, DIM), dtype=np.float32)
    h_fin = np.empty((B, DS), dtype=np.float32)
    for b in range(B):
        for h in range(2):
            orec = np.asarray(res.results[b][f"out_rec{h}"]).reshape(
                P, OG, K, CH)
            out[b, h * TH:(h + 1) * TH] = orec.transpose(3, 2, 1, 0).reshape(
                TH, DIM)
        h_fin[b] = np.asarray(res.results[b]["h_fin"]).reshape(P, SG).T.reshape(DS)
    return out, h_fin


# revision 27
# speedup vs baseline: 1.1303x; 1.0093x over previous
"""DiagonalStateElman Trainium2 kernel.

Math (per batch row b, data-parallel over 8 cores):
    xz   = x @ W_in^T                       -> split x_proj, z
    u_t  = silu(x_proj_t) @ B_mat           (h-independent, precomputable)
    h_t  = tanh(A*h_{t-1} + u_t)            (elementwise, A = sigmoid(log_A))
    y_t  = (h_t @ C_mat) * silu(z_t)
    out  = y @ W_out^T

Key trick: A = sigmoid(log_A) < 1 and |tanh'| <= 1, so the recurrence forgets
its state geometrically (factor max(A) per step).  T is split into chunks of
K=64 tokens; every chunk restarts from zero state W=32 tokens early
(max(A)^W < 2^-26, below fp32 resolution) so all chunks of a T/2 half run in
lockstep: 96 sequential steps of one wide vector op instead of 2048.

The recurrence of each half overlaps the tensor engine's projection work of
the other half, so wall time ~= pure matmul time.  Everything is transposed
(features on partitions, time on free axis) so the four matmuls chain without
transposes.  fp16 operands, fp32 PSUM accumulate; the u buffer is stored
k-major so the per-step strided gather is contiguous.
"""

import numpy as np

import concourse.bass as bass
import concourse.tile as tile
import concourse.bacc as bacc
import concourse.mybir as mybir
from concourse.bass_utils import run_bass_kernel_spmd

P = 128
T = 2048
DIM = 1024
DI = 1024          # d_inner
DS = 2048          # d_state
NCORES = 8
TS = 512           # time-slab for projection phases
K = 64             # chunk length
W = 32             # warmup tokens (also the front-pad inside block 0)
F16 = mybir.dt.float16
F32 = mybir.dt.float32

DG = DIM // P      # 8  dim groups (contraction for XZ)
EG = 2 * DI // P   # 16 e-chunks of XZ output
SG = DS // P       # 16 d_state groups
JG = DI // P       # 8  d_inner groups
OG = DIM // P      # 8  output dim groups
TH = T // 2        # tokens per half
CH = TH // K       # 16 chunks per half
NC_ = CH + 1       # u-buffer blocks per half (1 leading warmup block)
NSLAB = T // TS    # 4 slabs (2 per half)
STEPS = K + W      # 96 lockstep steps per half
IB = 32            # macro-2 step-block (IB*CH = 512 = one PSUM bank)

LAST_RESULT = None  # stashed BassKernelResults (for the test harness)


def _build(a_const):
    """Emit + compile the Bass program. a_const: float A (constant) or None."""
    mult = mybir.AluOpType.mult
    add = mybir.AluOpType.add
    Sigmoid = mybir.ActivationFunctionType.Sigmoid
    Tanh = mybir.ActivationFunctionType.Tanh

    nc = bacc.Bacc("TRN2", target_bir_lowering=False, debug=False,
                   num_devices=NCORES)

    xT_d = nc.dram_tensor("xT", [DIM, T], F16, kind="ExternalInput").ap()
    w_inT_d = nc.dram_tensor("w_inT", [DIM, 2 * DI], F16, kind="ExternalInput").ap()
    b_d = nc.dram_tensor("b_mat", [DI, DS], F16, kind="ExternalInput").ap()
    c_d = nc.dram_tensor("c_mat", [DS, DI], F16, kind="ExternalInput").ap()
    w_outT_d = nc.dram_tensor("w_outT", [DI, DIM], F16, kind="ExternalInput").ap()
    ah0_d = nc.dram_tensor("ah0", [P, SG], F32, kind="ExternalInput").ap()
    a_d = None
    if a_const is None:
        a_d = nc.dram_tensor("a_vec", [P, SG], F32, kind="ExternalInput").ap()
    out_hd = [nc.dram_tensor(f"out_rec{h}", [P, OG, K, CH], F32,
                             kind="ExternalOutput").ap() for h in range(2)]
    hfin_d = nc.dram_tensor("h_fin", [P, SG], F32, kind="ExternalOutput").ap()

    xT_v = xT_d.rearrange("(g p) t -> p g t", p=P)
    w_inT_v = w_inT_d.rearrange("(g p) e -> p g e", p=P)
    b_v = b_d.rearrange("(g p) s -> p g s", p=P)
    c_v = c_d.rearrange("(g p) j -> p g j", p=P)
    w_outT_v = w_outT_d.rearrange("(g p) o -> p g o", p=P)

    with tile.TileContext(nc) as tc:
        with tc.tile_pool(name="dram", bufs=1, space="DRAM") as dpool, \
             tc.tile_pool(name="psum", bufs=6, space="PSUM") as ppool, \
             tc.tile_pool(name="st", bufs=3) as st, \
             tc.tile_pool(name="hst", bufs=2) as hst, \
             tc.tile_pool(name="u1p", bufs=1) as u1p, \
             tc.tile_pool(name="ahp", bufs=1) as ahp:

            gz_dd = [dpool.tile([P, JG, TH], F16, tag=f"gz_d{h}",
                                name=f"gz_d{h}") for h in range(2)]
            # H in recurrence order, 8-step blocks: [p, g, i8, c]
            hrec = [[dpool.tile([P, SG, 8, CH], F16, tag=f"hrec{h}_{i}",
                                name=f"hrec{h}_{i}") for i in range(K // 8)]
                    for h in range(2)]

            ah0_sb = ahp.tile([P, SG], F32, tag="ah0")
            nc.scalar.dma_start(out=ah0_sb, in_=ah0_d)
            a_sb = None
            if a_const is None:
                a_sb = ahp.tile([P, SG], F32, tag="a_sb")
                nc.scalar.dma_start(out=a_sb, in_=a_d)

            # u buffers, k-major: [p, k, g, c_blk]; col (c_blk*K + k) holds
            # u(token_local = c_blk*K + k - K); block 0 k>=W is the leading
            # warmup pad of the half.  U[1] outlives the macro-1 pools (read
            # by the half-1 recurrence that overlaps macro 2).
            U = [None, u1p.tile([P, K, SG, NC_], F16, tag="U1", name="U1")]

            # ---------------- projection phases (per 512-token slab) -------
            def emit_slab(sl, ws, xs, sxp, gzp, sgp, b_sb):
                h = sl // 2
                sloc = sl % 2
                t0 = sl * TS
                xslab = xs.tile([P, DG, TS], F16, tag="x", name=f"x{sl}")
                nc.sync.dma_start(out=xslab, in_=xT_v[:, :, t0:t0 + TS])
                sxs = sxp.tile([P, DG, TS], F16, tag="sx", name=f"sx{sl}")
                gzs = gzp.tile([P, JG, TS], F16, tag="gz", name=f"gz{sl}")
                for ec2 in range(EG // 2):
                    wt = ws.tile([P, DG, 2 * P], F16, tag="w", name=f"w{sl}_{ec2}")
                    nc.sync.dma_start(
                        out=wt, in_=w_inT_v[:, :, ec2 * 2 * P:(ec2 + 1) * 2 * P])
                    for half in range(2):
                        ec = ec2 * 2 + half
                        ps = ppool.tile([P, TS], F32, tag="mm", name=f"psA{sl}_{ec}")
                        for g in range(DG):
                            nc.tensor.matmul(
                                ps, lhsT=wt[:, g, half * P:(half + 1) * P],
                                rhs=xslab[:, g, :],
                                start=(g == 0), stop=(g == DG - 1))
                        tgt = sxs[:, ec, :] if ec < DG else gzs[:, ec - DG, :]
                        # silu(v) = v * sigmoid(v)
                        sg = sgp.tile([P, TS], F16, tag="sg", name=f"sg{sl}_{ec}")
                        nc.scalar.activation(out=sg, in_=ps, func=Sigmoid)
                        nc.vector.tensor_tensor(out=tgt, in0=ps, in1=sg, op=mult)
                nc.sync.dma_start(
                    out=gz_dd[h][:, :, sloc * TS:(sloc + 1) * TS], in_=gzs)
                for sc in range(SG):
                    ps = ppool.tile([P, TS], F32, tag="mm", name=f"psB{sl}_{sc}")
                    for g in range(DG):
                        nc.tensor.matmul(
                            ps, lhsT=b_sb[:, g, sc * P:(sc + 1) * P],
                            rhs=sxs[:, g, :],
                            start=(g == 0), stop=(g == DG - 1))
                    # transposed store: psum col j=(c*K+k) -> U[k, sc, c0+c]
                    c0 = 1 + sloc * (TS // K)
                    nc.vector.tensor_copy(
                        out=U[h][:, :, sc, c0:c0 + TS // K],
                        in_=ps.rearrange("p (c k) -> p k c", k=K))
                    if sl == 1:
                        # warmup pad of half 1 = last W tokens of half 0
                        nc.vector.tensor_copy(
                            out=U[1][:, W:K, sc, 0], in_=ps[:, TS - W:TS])

            # -------------- lockstep chunked recurrence for one half -------
            def emit_rec(h):
                GH = SG // 2
                Sc = []
                for ch in range(2):
                    s0 = st.tile([P, GH, CH], F16, tag=f"s{ch}",
                                 name=f"s0_{h}_{ch}")
                    nc.vector.memset(s0, 0.0)
                    Sc.append(s0)
                hstage = None
                for i in range(STEPS):
                    r = i - W
                    if r >= 0 and r % 8 == 0:
                        hstage = hst.tile([P, SG, 8, CH], F16, tag="hst",
                                          name=f"hs{h}_{r // 8}")
                    kk = i + W
                    c_off = kk // K
                    k = kk % K
                    for ch in range(2):
                        g0 = ch * GH
                        u = U[h][:, k, g0:g0 + GH, c_off:c_off + CH]
                        t1 = st.tile([P, GH, CH], F32, tag=f"t{ch}",
                                     name=f"t{h}_{i}_{ch}")
                        if a_const is not None:
                            nc.vector.scalar_tensor_tensor(
                                out=t1, in0=Sc[ch], scalar=a_const, in1=u,
                                op0=mult, op1=add)
                        else:
                            nc.vector.tensor_tensor(
                                out=t1, in0=Sc[ch],
                                in1=a_sb[:, g0:g0 + GH, None].to_broadcast(
                                    [P, GH, CH]), op=mult)
                            nc.vector.tensor_tensor(out=t1, in0=t1, in1=u, op=add)
                        if r >= 0:
                            s1 = hstage[:, g0:g0 + GH, r % 8, :]
                        else:
                            s1 = st.tile([P, GH, CH], F16, tag=f"s{ch}",
                                         name=f"sw{h}_{i}_{ch}")
                        nc.scalar.activation(out=s1, in_=t1, func=Tanh)
                        Sc[ch] = s1
                        if h == 1 and i == STEPS - 1:
                            hf = st.tile([P, GH, CH], F32, tag=f"hf{ch}",
                                         name=f"hf{ch}", bufs=1)
                            nc.scalar.activation(out=hf, in_=t1, func=Tanh)
                            nc.gpsimd.dma_start(
                                out=hfin_d[:, g0:g0 + GH],
                                in_=hf[:, :, CH - 1])
                    if r >= 0 and r % 8 == 7:
                        nc.sync.dma_start(out=hrec[h][r // 8], in_=hstage)

            # -------------- macro 2 for one half: Y, gate, out -------------
            def emit_m2(h, c_sb, wout_sb, gz_sb, hsp, gp, op_):
                for ib in range(K // IB):
                    hsl = hsp.tile([P, SG, IB, CH], F16, tag="hsl",
                                   name=f"hsl{h}_{ib}")
                    for q in range(IB // 8):
                        nc.sync.dma_start(
                            out=hsl[:, :, q * 8:(q + 1) * 8, :],
                            in_=hrec[h][ib * (IB // 8) + q])
                    hflat = hsl.rearrange("p g a b -> p g (a b)")
                    Gt = gp.tile([P, JG, IB * CH], F16, tag="G",
                                 name=f"G{h}_{ib}")
                    for jc in range(JG):
                        ps = ppool.tile([P, IB * CH], F32, tag="mm",
                                        name=f"psY{h}_{ib}_{jc}")
                        for g in range(SG):
                            nc.tensor.matmul(
                                ps, lhsT=c_sb[:, g, jc * P:(jc + 1) * P],
                                rhs=hflat[:, g, :],
                                start=(g == 0), stop=(g == SG - 1))
                        gzv = gz_sb[:, jc, :].rearrange(
                            "p (c i) -> p i c", i=K)[:, ib * IB:(ib + 1) * IB, :]
                        nc.vector.tensor_tensor(
                            out=Gt[:, jc, :].rearrange("p (a b) -> p a b", b=CH),
                            in0=ps.rearrange("p (a b) -> p a b", b=CH),
                            in1=gzv, op=mult)
                    for oc in range(OG):
                        ps2 = ppool.tile([P, IB * CH], F32, tag="mm",
                                         name=f"psO{h}_{ib}_{oc}")
                        for jc in range(JG):
                            nc.tensor.matmul(
                                ps2, lhsT=wout_sb[:, jc, oc * P:(oc + 1) * P],
                                rhs=Gt[:, jc, :],
                                start=(jc == 0), stop=(jc == JG - 1))
                        ost = op_.tile([P, IB * CH], F32, tag="ost",
                                       name=f"ost{h}_{ib}_{oc}")
                        nc.vector.tensor_copy(out=ost, in_=ps2)
                        nc.gpsimd.dma_start(
                            out=out_hd[h][:, oc, ib * IB:(ib + 1) * IB, :],
                            in_=ost.rearrange("p (a b) -> p a b", b=CH))

            # ------------------------- schedule ----------------------------
            with tc.tile_pool(name="m2w", bufs=1) as m2w:
                with tc.tile_pool(name="ws", bufs=3) as ws, \
                     tc.tile_pool(name="xs", bufs=2) as xs, \
                     tc.tile_pool(name="sxp", bufs=1) as sxp, \
                     tc.tile_pool(name="gzp", bufs=1) as gzp, \
                     tc.tile_pool(name="sgp", bufs=2) as sgp, \
                     tc.tile_pool(name="u0p", bufs=1) as u0p, \
                     tc.tile_pool(name="m1", bufs=1) as m1:
                    U[0] = u0p.tile([P, K, SG, NC_], F16, tag="U0", name="U0")
                    nc.vector.memset(U[0][:, W:K, :, 0], 0.0)
                    b_sb = m1.tile([P, DG, DS], F16, tag="b_sb")
                    nc.scalar.dma_start(out=b_sb, in_=b_v)

                    emit_slab(0, ws, xs, sxp, gzp, sgp, b_sb)
                    emit_slab(1, ws, xs, sxp, gzp, sgp, b_sb)
                    # gz for half 0: load back early, hidden under slab 2/3
                    gz_sb0 = m2w.tile([P, JG, TH], F16, tag="gzsb0",
                                      name="gzsb0")
                    nc.sync.dma_start(out=gz_sb0, in_=gz_dd[0])
                    # exact h0 handling: u(token 0) += A*h0  (col K of half 0)
                    nc.vector.tensor_tensor(
                        out=U[0][:, 0, :, 1], in0=U[0][:, 0, :, 1], in1=ah0_sb,
                        op=add)
                    emit_rec(0)
                    emit_slab(2, ws, xs, sxp, gzp, sgp, b_sb)
                    # macro-2 C weights load while slabs 2/3 + rec-h0 run
                    c_sb = m2w.tile([P, SG, DI], F16, tag="c_sb")
                    nc.scalar.dma_start(out=c_sb, in_=c_v)
                    emit_slab(3, ws, xs, sxp, gzp, sgp, b_sb)

                with tc.tile_pool(name="hsp", bufs=2) as hsp, \
                     tc.tile_pool(name="gp", bufs=2) as gp, \
                     tc.tile_pool(name="op", bufs=3) as op_, \
                     tc.tile_pool(name="m2b", bufs=1) as m2b:
                    wout_sb = m2b.tile([P, JG, DIM], F16, tag="wout_sb")
                    nc.scalar.dma_start(out=wout_sb, in_=w_outT_v)
                    emit_m2(0, c_sb, wout_sb, gz_sb0, hsp, gp, op_)
                    gz_sb1 = m2b.tile([P, JG, TH], F16, tag="gzsb1",
                                      name="gzsb1")
                    nc.sync.dma_start(out=gz_sb1, in_=gz_dd[1])
                    emit_rec(1)
                    emit_m2(1, c_sb, wout_sb, gz_sb1, hsp, gp, op_)

    nc.compile()
    return nc


def kernel(x, h0, W_in, W_out, B_mat, C_mat, log_A):
    global LAST_RESULT
    x = np.asarray(x, dtype=np.float32)
    h0 = np.asarray(h0, dtype=np.float32)
    W_in = np.asarray(W_in, dtype=np.float32)
    W_out = np.asarray(W_out, dtype=np.float32)
    B_mat = np.asarray(B_mat, dtype=np.float32)
    C_mat = np.asarray(C_mat, dtype=np.float32)
    log_A = np.asarray(log_A, dtype=np.float32)

    B = x.shape[0]
    assert x.shape == (NCORES, T, DIM), x.shape

    A = 1.0 / (1.0 + np.exp(-log_A.astype(np.float64)))
    A = A.astype(np.float32)
    a_const = float(A[0]) if np.all(A == A[0]) else None
    amax = float(A.max())
    if amax > 0.0 and (amax >= 1.0 or -26.0 / np.log2(amax) > W):
        raise NotImplementedError(
            f"A too close to 1 for warmup {W} (amax={amax})")

    nc = _build(a_const)

    w_inT16 = np.ascontiguousarray(W_in.T).astype(np.float16)
    b16 = np.ascontiguousarray(B_mat).astype(np.float16)
    c16 = np.ascontiguousarray(C_mat).astype(np.float16)
    w_outT16 = np.ascontiguousarray(W_out.T).astype(np.float16)

    in_maps = []
    for b in range(B):
        m = {
            "xT": np.ascontiguousarray(x[b].T).astype(np.float16),
            "w_inT": w_inT16,
            "b_mat": b16,
            "c_mat": c16,
            "w_outT": w_outT16,
            "ah0": np.ascontiguousarray((A * h0[b]).reshape(SG, P).T).astype(
                np.float32),
        }
        if a_const is None:
            m["a_vec"] = np.ascontiguousarray(A.reshape(SG, P).T).astype(
                np.float32)
        in_maps.append(m)

    res = run_bass_kernel_spmd(nc, in_maps, core_ids=list(range(NCORES)))
    LAST_RESULT = res

    out = np.empty((B, T, DIM), dtype=np.float32)
    h_fin = np.empty((B, DS), dtype=np.float32)
    for b in range(B):
        for h in range(2):
            orec = np.asarray(res.results[b][f"out_rec{h}"]).reshape(
                P, OG, K, CH)
            out[b, h * TH:(h + 1) * TH] = orec.transpose(3, 2, 1, 0).reshape(
                TH, DIM)
        h_fin[b] = np.asarray(res.results[b]["h_fin"]).reshape(P, SG).T.reshape(DS)
    return out, h_fin
